# revision 23
# baseline (speedup 1.0000x reference)
"""Trainium2 Bass kernel for nn_Block_47545287967557 (dense_cnn).

The reference module, simplified:
  - dead avgpool->linear->relu path (result unused)
  - sum over K=4 conv branches == ONE 3x3 VALID conv with weights Wc.sum(0)
    and bias bc.sum(0):  O[b,co,y,x] = sum_{ci,dy,dx} Weff[co,ci,dy,dx] *
    X[b,ci,y+dy,x+dx] + beff[co]
  X: [32,3,512,512] fp32 -> O: [32,3,510,510] fp32.

Strategy: pure data-parallel over batch across 8 NeuronCores (4 images each).
Per core the conv runs on the tensor engine as block-banded matmuls:
  contraction K = (c_in, yi) packed into 126 partitions (42-row y window)
  plus a constant-ones row 126 that carries the bias (stationary row 126 of
  the dx=0 matrix holds beff, so PSUM comes out pre-biased), output
  M = (c_out, yo) packed into 120 partitions (+8 zero pad to 128 for FWL),
  moving N = 510 x positions; one matmul per dx shift (3, PSUM-accumulated).
  13 y-blocks per image (y0 = 0,40,...,440,470; the last overlaps rows
  470..479 with identical values).

Precision/bandwidth: X is cast to fp16 on the HOST and DMA'd as fp16 (the
matmul runs in fp16; this halves input HBM traffic). fp8 was measured (CPU
sim, exact seed-0 data): e4m3 rel err 4.0e-2, e3m4 2.97e-2 -- both over the
2e-2 gate, so fp16 it is (3.4e-4). The PSUM->SBUF downcast copy alternates
between the scalar and vector engines (a single engine would serialize at
~35us). Output is stored fp16 (host upcasts to fp32 while unsharding).

DMA: trn2 has two HWDGE rings, FIFO per ring (sync/SP and scalar/Act).
Inputs stream on the SP ring, outputs + consts on the Act ring so reads and
writes overlap. Outputs of earlier images must NOT use the SP ring: a
sequencer issues in program order, so an output trigger there would block
later input triggers behind the compute dependency. Only the last image's
output is split across both rings (4 chunks) to halve the drain tail.
The stationary matrices load as ONE DMA [127, 3*128] (127 x 768B
descriptors, ~2.4us) -- as 3+1 separate tensors they were 555 x 256B
descriptors taking ~10us, stalling the first image's dx=1/2 matmuls.
"""

import sys

sys.path.insert(0, "/opt/trn_rl_repo")

import numpy as np

N_CORES = 8
B_PER_CORE = 4
C = 3
H = W = 512
OH = OW = 510
NBLK = 13
KP = C * 42 + 1  # 126 contraction partitions + ones row (bias)
MP = C * 40      # 120 live output partitions
MPAD = 128       # stationary columns padded for FWL
IN_CHUNKS = {
    0: [(0, 2), (2, 7), (7, 13)],   # small first chunk -> compute starts early
    1: [(0, 13)], 2: [(0, 13)], 3: [(0, 13)],  # whole-image DMAs, 13KB descs
}
OUT_CHUNKS = {
    0: [(0, 7), (7, 13)],
    1: [(0, 7), (7, 13)],
    2: [(0, 7), (7, 13)],
    3: [(0, 4), (4, 7), (7, 9), (9, 10), (10, 11), (11, 12), (12, 13)],
}
WARMUP_MM = 12  # dummy matmuls to ramp the PE clock before real data lands

_CACHE = {}


def _build_weights(Wc, bc):
    Weff = np.asarray(Wc, dtype=np.float32).sum(axis=0)  # [co, ci, dy, dx]
    beff = np.asarray(bc, dtype=np.float32).sum(axis=0)  # [co]
    S = np.zeros((MPAD, 3, MPAD), dtype=np.float32)  # 128 rows: even SDMA split
    for dx in range(3):
        for c_in in range(C):
            for c_out in range(C):
                for yo in range(40):
                    for dy in range(3):
                        S[c_in * 42 + yo + dy, dx, c_out * 40 + yo] = Weff[c_out, c_in, dy, dx]
    # bias rides the ones-row through the dx=0 (start) matmul
    for c_out in range(C):
        S[C * 42, 0, c_out * 40:(c_out + 1) * 40] = beff[c_out]
    return S.astype(np.float16)


def _build_program():
    import concourse.bass as bass
    import concourse.mybir as mybir
    import concourse.tile as tile
    from concourse import bacc

    nc = bacc.Bacc("TRN2", target_bir_lowering=False, debug=False)

    # XS carries only the 126 data rows: a 127-partition DMA cannot be split
    # evenly across the SDMA engines (127 is prime) and lands on ONE engine
    # at ~26GB/s (measured). The ones-row is a separate 1-descriptor DMA.
    XS = nc.dram_tensor("XS", [B_PER_CORE, KP - 1, NBLK, W], mybir.dt.float16, kind="ExternalInput")
    ONES = nc.dram_tensor("ONES", [1, NBLK, W], mybir.dt.float16, kind="ExternalInput")
    SMAT = nc.dram_tensor("SMAT", [MPAD, 3, MPAD], mybir.dt.float16, kind="ExternalInput")
    OUT = nc.dram_tensor("OUT", [B_PER_CORE, MP, NBLK, OW], mybir.dt.float16, kind="ExternalOutput")

    f32 = mybir.dt.float32
    f16 = mybir.dt.float16

    with tile.TileContext(nc) as tc:
        with (
            tc.tile_pool(name="consts", bufs=1) as consts,
            tc.tile_pool(name="xs", bufs=4) as xpool,
            tc.tile_pool(name="os", bufs=4) as opool,
            tc.tile_pool(name="ps", bufs=7, space=bass.MemorySpace.PSUM) as ppool,
        ):
            # one DMA, 128 descriptors of 768B, on the Act ring (the SP ring
            # belongs to the input stream)
            smat_t = consts.tile([MPAD, 3, MPAD], f16, tag="smat")
            nc.scalar.dma_start(out=smat_t[:], in_=SMAT.ap())

            # PE p-state warmup: the tensor clock ramps 0.65 -> 2.4GHz only
            # after ~3us of continuous work, so the first real matmuls would
            # run at half speed. Burn dummy matmuls on an uninitialized
            # scratch tile into a scratch PSUM bank while the input DMA is in
            # flight (start=stop=True, result never read; a later start=True
            # reset would clear any NaN garbage anyway -- this bank is unused).
            wsrc = consts.tile([MPAD, W], f16, tag="warmup_src")
            nc.gpsimd.memset(wsrc[:], 1.0)
            wp = ppool.tile([MPAD, OW], f32, bufs=1)
            for _ in range(WARMUP_MM):
                nc.tensor.matmul(wp[:], wsrc[:, 0:MPAD], wsrc[:, 0:OW], start=True,
                                 stop=True, skip_group_check=True)

            # Phase A: queue the ENTIRE input stream on the SP ring up front
            # (all 4 images stay resident in SBUF -- ~6.7MB). Nothing else may
            # queue on this ring mid-stream: the sequencer is FIFO, so a
            # compute-dependent trigger would stall everything behind it.
            xbs = []
            for img in range(B_PER_CORE):
                xb = xpool.tile([KP, NBLK, W], f16)
                nc.sync.dma_start(out=xb[KP - 1:KP, :, :], in_=ONES.ap())
                for b0, b1 in IN_CHUNKS[img]:
                    nc.sync.dma_start(out=xb[:KP - 1, b0:b1, :], in_=XS.ap()[img, :, b0:b1, :])
                xbs.append(xb)

            # Phase B: compute + copies; outputs alternate across BOTH rings
            # (the SP ring drains its queued inputs first, then serves its
            # share of outputs -- by then they are ready anyway).
            copy_idx = 0
            out_idx = 0
            for img in range(B_PER_CORE):
                xb = xbs[img]
                ot = opool.tile([MP, NBLK, OW], f16)
                for b0, b1 in OUT_CHUNKS[img]:
                    for b in range(b0, b1):
                        pt = ppool.tile([MPAD, OW], f32)
                        for dx in range(3):
                            nc.tensor.matmul(
                                pt[:],
                                smat_t[0:KP, dx, :],
                                xb[:, b, dx:dx + OW],
                                start=(dx == 0),
                                stop=(dx == 2),
                            )
                        # PSUM -> SBUF downcast (bias already in PSUM),
                        # alternating between the two PSUM-capable engines
                        if copy_idx % 2 == 0:
                            nc.scalar.copy(ot[:, b, :], pt[0:MP, :])
                        else:
                            nc.vector.tensor_copy(ot[:, b, :], pt[0:MP, :])
                        copy_idx += 1
                    if out_idx % 2 == 0:
                        nc.scalar.dma_start(out=OUT.ap()[img, :, b0:b1, :], in_=ot[:, b0:b1, :])
                    else:
                        nc.sync.dma_start(out=OUT.ap()[img, :, b0:b1, :], in_=ot[:, b0:b1, :])
                    out_idx += 1

    nc.compile()
    return nc


def _get_nc():
    if "nc" not in _CACHE:
        _CACHE["nc"] = _build_program()
    return _CACHE["nc"]


def run_spmd(in_maps, **kwargs):
    from concourse.bass_utils import run_bass_kernel_spmd

    nc = _get_nc()
    return run_bass_kernel_spmd(nc, in_maps, list(range(N_CORES)), **kwargs)


def make_in_maps(X, Wc, bc):
    X = np.ascontiguousarray(np.asarray(X, dtype=np.float32))
    Sb = _build_weights(Wc, bc)

    # overlap-window shard: XP[core, img, c*42+yi, b, x] = X[4*core+img, c, y0(b)+yi, x]
    Xr = X.reshape(N_CORES, B_PER_CORE, C, H, W)
    XP = np.empty((N_CORES, B_PER_CORE, KP - 1, NBLK, W), dtype=np.float16)
    XPw = XP.reshape(N_CORES, B_PER_CORE, C, 42, NBLK, W)
    s = Xr.strides
    win = np.lib.stride_tricks.as_strided(
        Xr, shape=(N_CORES, B_PER_CORE, C, 12, 42, W),
        strides=(s[0], s[1], s[2], 40 * s[3], s[3], s[4]))
    XPw[:, :, :, :, 0:12, :] = win.transpose(0, 1, 2, 4, 3, 5)
    XPw[:, :, :, :, 12, :] = Xr[:, :, :, 470:512, :]
    ones = np.ones((1, NBLK, W), dtype=np.float16)

    return [
        {"XS": XP[i], "ONES": ones, "SMAT": Sb}
        for i in range(N_CORES)
    ]


def gather_output(res):
    """[core][img, (c,yo), b, x] -> [32, 3, 510, 510]"""
    OUTP = np.stack([res.results[i]["OUT"] for i in range(N_CORES)]).astype(np.float32)
    R = OUTP.reshape(N_CORES, B_PER_CORE, C, 40, NBLK, OW)
    O = np.empty((N_CORES, B_PER_CORE, C, OH, OW), dtype=np.float32)
    O[:, :, :, 0:480, :] = (
        R[:, :, :, :, 0:12, :].transpose(0, 1, 2, 4, 3, 5).reshape(N_CORES, B_PER_CORE, C, 480, OW)
    )
    O[:, :, :, 480:OH, :] = R[:, :, :, 10:40, 12, :]
    return O.reshape(N_CORES * B_PER_CORE, C, OH, OW)


def kernel(X, Wc, bc, linW, linb):
    res = run_spmd(make_in_maps(X, Wc, bc))
    return gather_output(res)
